# revision 66
# baseline (speedup 1.0000x reference)
"""Trainium2 Bass kernel for the DSIB InfoNCE loss (fp8 DoubleRow version).

Reference computation (B=512, NX=NY=64, HID=256):
    scores[i, j] = MLP(concat(x_j, y_i))       # 3-layer MLP, scalar out
    loss = -(log B + mean(diag(scores)) - mean(logsumexp(scores, axis=1)))

Strategy (data-parallel over the outer y index, 8 cores x 64 rows):
  * Layer 1 is linear in the concatenation: precompute on device
    A = X @ W1[:64] ([512, 256], kept transposed as a16 [128, 2, 512] fp16)
    and C = Y_shard @ W1[64:] + b1 (cb [128, 2, 64] fp32, b1 folded in via
    an augmented ones-row on the host). h1(i, j) = relu(A[j] + C[i]).
  * Layer 2 runs in fp8e4m3 with perf_mode=DoubleRow: h1 is produced
    directly in fp8 as [128, 2, 512] (contraction 256 = 128 partitions x 2
    packed), W2 is stochastically-rounded to fp8 on the host and the
    systematic quantization error is compensated by b2' = b2 - dW2.T @ mean(h1)
    (also computed host-side).  One DoubleRow matmul per (row, out-half)
    does the whole K=256 contraction at 2 MACs/cell/cycle.
  * Layer-2 PSUM output is 2-row batched ([128, 1024] spanning 2 banks) so
    one ACT relu+bias pass covers two rows -> h2 [128, 2, 512] fp16.
  * The relu+bias pass (h2) is the bottleneck (1 elem/lane/cycle from
    PSUM on both ACT and DVE); per group 3 units run on ACT and 1 on DVE
    (empirically balanced against ACT's exp and DVE's h1 work).
  * Layer 3 (fp16) accumulates scores for 8 groups of 4 rows into one
    persistent PSUM bank: for group g the stationary w3 column sits at
    offset g%8 inside a 32-wide zero-padded block at column-group 32j, so
    row 4g+j lands at PSUM partition 32j + g%8.  Scores are never copied
    to SBUF: the per-half sumexp (ACT exp with accum_out; log on host)
    and the masked diagonal extraction (one fused scalar_tensor_tensor
    with accum_out) read the PSUM bank directly.  Each core returns
    [128, 4] = per-partition (sumexp, diag) x 2 halves in the permuted
    layout; the host unpermutes and reduces.
"""

import sys

import numpy as np

_TRN_REPO = "/opt/trn_rl_repo"
if _TRN_REPO not in sys.path:
    sys.path.insert(0, _TRN_REPO)

B = 512
NX = 64
NY = 64
HID = 256
N_CORES = 8
SH = B // N_CORES  # y rows per core
SR_SEED = 0  # host-side stochastic-rounding seed for W2 (validated on HW)

_PROG_CACHE = {}


def _emit(
    tc,
    aps,
    n_rows=SH,
    do_endgame=True,
    repeat=None,
    variant="full",  # full | no_l3 | no_h1 | no_l2 | l2_only
    h1_act_cols=0,  # leading h1 columns computed on ACT instead of DVE
    h2_dve=1,  # how many of the 4 h2 units per group run on DVE vs ACT
    h2_extra_dve=False,  # additionally put unit 1 on DVE for even groups
    h2_qsplit=False,  # split DVE h2 units into [128, 256] quarter ops
    h2_pool=0,  # how many of the 4 h2 units per group run on Pool/GPSIMD
    mm_order="m",  # "m": m-major (stationary reused), "w": alternating
    l3_deps=False,  # fake deps grouping the L3 wave (measured: hurts here)
    h1_ahead=True,  # software-pipeline h1 one group ahead of the DVE h2 unit
    h2_d_cols=768,  # DVE's h2 unit covers cols [0:D]; ACT takes [D:1024]
    h2_u1_cols=0,  # if >0, DVE additionally takes cols [S:1024] of unit 1
):
    _H2_DVE_SETS = {
        0: set(),
        1: {3},
        2: {1, 3},
        3: {1, 2, 3},
        4: {0, 1, 2, 3},
        "u0": {0},
        "u1": {1},
        "u2": {2},
    }
    h2_dve_units = _H2_DVE_SETS[h2_dve]
    _H2_POOL_SETS = {0: set(), 1: {1}, 2: {1, 2}}
    h2_pool_units = _H2_POOL_SETS[h2_pool]
    import contextlib

    import concourse.bass as bass  # noqa: F401
    from concourse import mybir
    from concourse.tile import add_dep_helper

    nc = tc.nc
    f32 = mybir.dt.float32
    f32r = mybir.dt.float32r
    f16 = mybir.dt.float16
    f8 = mybir.dt.float8e4
    AF = mybir.ActivationFunctionType
    ALU = mybir.AluOpType
    AX = mybir.AxisListType
    DR = mybir.MatmulPerfMode.DoubleRow

    xt_d = aps["xt"]
    yta_d = aps["yta"]
    w1x_d = aps["w1x"]
    w1ya_d = aps["w1ya"]
    w2_d = aps["w2"]
    b2_d = aps["b2"]
    w3s_d = aps["w3s"]
    mask_d = aps["mask"]
    out_d = aps["out"]

    assert n_rows % 8 == 0, "row loop works in half-kernels of 8 groups"
    n_groups = n_rows // 4
    rows_per_half = 4 * min(8, n_groups)

    with (
        tc.tile_pool(name="const", bufs=1) as cpool,
        tc.tile_pool(name="work", bufs=2) as wpool,
        tc.tile_pool(name="psum", bufs=3, space="PSUM") as ppool,
        tc.tile_pool(name="psall", bufs=1, space="PSUM") as papool,
    ):
        psall = (
            [
                papool.tile([128, B], f32, tag=f"psall{h}", name=f"psall{h}")
                for h in range(2 if n_groups > 8 else 1)
            ]
            if variant in ("full", "no_h1")
            else []
        )

        # ---------------- persistent loads ----------------
        # Critical-path tensors (layer 1) on the SP HWDGE queue, in order;
        # everything else on the Activation HWDGE queue, so the two rings
        # drain in parallel.
        # layer-1-critical tensors first on the SP HWDGE queue; the compute
        # that needs them is emitted immediately after, so the scheduler
        # splits the DMA-completion semaphores there (emission order sets
        # priority).  Bulk tensors ride the Activation HWDGE ring.
        # xt split across both HWDGE rings to halve the first-DMA wall
        xt = cpool.tile([NX, B], f32r, name="xt_sb")
        nc.sync.dma_start(xt[0:32, :], xt_d[0:32, :])
        nc.scalar.dma_start(xt[32:64, :], xt_d[32:64, :])
        w1x = cpool.tile([NX, HID], f32r, name="w1x_sb")
        nc.sync.dma_start(w1x[:], w1x_d[:])

        # Dummy first ACT op: forces the one-time activation-table load to
        # start at t=0, overlapped with the input DMAs and layer 1.  Exp so
        # the chosen set (exp_and_others) also covers Relu/Identity -> one
        # table load for the whole kernel.
        actwarm = cpool.tile([1, 2], f32, name="actwarm")
        nc.gpsimd.memset(actwarm[:, 0:1], 0.0)
        nc.scalar.activation(actwarm[:, 1:2], actwarm[:, 0:1], AF.Exp)

        # HAM warm-up: ~2us of dummy matmuls on zeros while the input DMAs
        # land, so the PE clock gate reaches 8/8 before layer 1/2 start.
        # They write the psall bank, which the first real L3 matmul clears
        # (start=True), so the garbage never escapes.
        if variant in ("full", "no_h1") and n_groups >= 1:
            zst = cpool.tile([128, 32], f8, name="zstat")
            nc.gpsimd.memset(zst[:], 0.0)
            jmov = cpool.tile([128, B], f8, name="jmov")
            nc.gpsimd.memset(jmov[:], 0.0)
            for _wi in range(5):
                nc.tensor.matmul(
                    psall[0][0:32, :],
                    zst[:],
                    jmov[:],
                    start=True,
                    stop=True,
                    tile_position=(0, 0),
                    skip_group_check=True,
                )

        # ---------------- layer-1 precompute (A side) ----------------
        pa = ppool.tile([128, 2, B], f32, tag="p2", name="pa")
        for m in range(2):
            nc.tensor.matmul(
                pa[:, m, :],
                w1x[:, 128 * m : 128 * m + 128],
                xt[:],
                start=True,
                stop=True,
            )
        # a16 cast split across ACT/DVE so each chunk lands in parallel and
        # the first h1 ops can start on chunk granularity
        a16 = cpool.tile([128, 2, B], f16, name="a16_sb")
        nc.scalar.copy(a16[:, 0, :], pa[:, 0, :])
        nc.vector.tensor_copy(a16[:, 1, :], pa[:, 1, :])

        # ---------------- remaining loads ----------------
        yta = cpool.tile([NY + 1, SH], f32r, name="yta_sb")
        nc.scalar.dma_start(yta[:], yta_d[:])
        w1ya = cpool.tile([NY + 1, HID], f32r, name="w1ya_sb")
        nc.scalar.dma_start(w1ya[:], w1ya_d[:])
        # W2 fp8: (p, k, m) = W2q[k*128 + p, m]
        w2 = cpool.tile([128, 2, HID], f8, name="w2_sb")
        for k in range(2):
            nc.sync.dma_start(w2[:, k, :], w2_d[128 * k : 128 * (k + 1), :])
        b2c = cpool.tile([128, 2], f32, name="b2_sb")
        nc.scalar.dma_start(b2c[:], b2_d.rearrange("(k p) -> p k", p=128))
        # w3s built on device: [128, g, k, 32] fp16, w3 chunk k at col g of
        # block g.  Load the compact [128, 2] w3 chunks, memset the block,
        # then 16 single-column SBUF->SBUF DMA copies (rings are idle).
        w3c = cpool.tile([128, 2], f16, name="w3c_sb")
        nc.scalar.dma_start(w3c[:], w3s_d[:, 0:2])
        w3s = cpool.tile([128, 8, 2, 32], f16, name="w3s_sb")
        nc.gpsimd.memset(w3s[:], 0.0)
        # scatter w3 chunk k to columns g*65 + k*32 (g=0..7) of the flat
        # [128, 512] block in one strided DVE copy per chunk (src g-dim is
        # stride-0)
        for k in range(2):
            src = bass.AP(w3c[:].tensor, k, [[2, 128], [0, 8]])
            dst = bass.AP(w3s[:].tensor, 32 * k, [[512, 128], [65, 8]])
            nc.vector.tensor_copy(dst, src)
        # per-half diag masks in the permuted psall layout (partition 32j+gg
        # holds row 32h + 4gg + j)
        mask = cpool.tile([128, 2, B], f8, name="mask_sb")
        nc.scalar.dma_start(mask[:], mask_d.rearrange("p (h c) -> p h c", h=2))

        # ---------------- layer-1 precompute (C side) ----------------
        pc = ppool.tile([128, 2, B], f32, tag="p2", name="pc")
        for m in range(2):
            nc.tensor.matmul(
                pc[:, m, 0:SH],
                w1ya[:, 128 * m : 128 * m + 128],
                yta[:],
                start=True,
                stop=True,
            )
        cb = cpool.tile([128, 2, SH], f32, name="cb_sb")
        nc.vector.tensor_copy(cb[:, 0, :], pc[:, 0, 0:SH])
        nc.vector.tensor_copy(cb[:, 1, :], pc[:, 1, 0:SH])

        if variant == "no_h1":
            h1d = cpool.tile([128, 2, B], f8, name="h1dummy")
            nc.vector.tensor_copy(h1d[:, 0, :], a16[:, 0, :])
            nc.vector.tensor_copy(h1d[:, 1, :], a16[:, 1, :])

        otile = cpool.tile([128, 4], f32, name="otile")

        # ---------------- main loop ----------------
        loop_cm = (
            tc.For_i(0, repeat, 1)
            if repeat is not None and repeat > 1
            else contextlib.nullcontext()
        )
        SPL = h1_act_cols

        def emit_h1_group(g):
            """h1 tiles for the 4 rows of group g (fp8, DoubleRow layout)."""
            if variant == "no_h1":
                return [h1d] * 4
            hs = []
            for q in range(2):
                for w in range(2):
                    i = 4 * g + 2 * q + w
                    h1 = wpool.tile(
                        [128, 2, B], f8, tag=f"h1_{q}_{w}", name=f"h1_{i}"
                    )
                    for k in range(2):
                        if SPL > 0:
                            nc.scalar.activation(
                                h1[:, k, 0:SPL],
                                a16[:, k, 0:SPL],
                                AF.Relu,
                                bias=cb[:, k, i : i + 1],
                            )
                        if SPL < B:
                            nc.vector.tensor_scalar(
                                h1[:, k, SPL:],
                                a16[:, k, SPL:],
                                cb[:, k, i : i + 1],
                                0.0,
                                ALU.add,
                                ALU.max,
                            )
                    hs.append(h1)
            return hs

        with loop_cm:
            halves = []  # half indices with completed endgame
            h1_next = emit_h1_group(0) if (n_groups and h1_ahead) else None
            for g in range(n_groups):
                ph = psall[g // 8] if psall else None
                goff = g % 8
                h1cur = h1_next if h1_ahead else emit_h1_group(g)
                h2g = []
                relus = []
                p2g = []
                if variant != "no_l2":
                    for q in range(2):  # row-pair within group
                        pr = 4 * g + 2 * q
                        p2s = [
                            ppool.tile(
                                [128, 2, B], f32, tag="p2", name=f"p2_{m}_{pr}"
                            )
                            for m in range(2)
                        ]
                        mw = (
                            [(0, 0), (0, 1), (1, 0), (1, 1)]
                            if mm_order == "m"
                            else [(0, 0), (1, 0), (0, 1), (1, 1)]
                        )
                        for m, w in mw:
                            nc.tensor.matmul(
                                p2s[m][:, w, :],
                                w2[:, :, 128 * m : 128 * (m + 1)],
                                h1cur[2 * q + w][:],
                                start=True,
                                stop=True,
                                perf_mode=DR,
                            )
                        p2g.append(p2s)

                # pipelined h1 for the NEXT group: emitted before this
                # group's DVE h2 unit so the (FIFO) DVE stream produces h1
                # ahead of chewing the long PSUM read -- keeps PE/ACT fed.
                if h1_ahead:
                    h1_next = emit_h1_group(g + 1) if g + 1 < n_groups else None

                if variant in ("no_l2", "l2_only"):
                    continue

                for q in range(2):
                    pr = 4 * g + 2 * q
                    for m in range(2):
                        h2 = wpool.tile(
                            [128, 2, B], f16, tag=f"h2_{q}_{m}", name=f"h2_{m}_{pr}"
                        )
                        unit = 2 * q + m
                        on_dve = unit in h2_dve_units or (
                            h2_extra_dve and unit == 1 and g % 2 == 0
                        )
                        h2f = h2[:].rearrange("p a b -> p (a b)")
                        p2f = p2g[q][m][:].rearrange("p a b -> p (a b)")
                        if unit in h2_pool_units and not on_dve:
                            r = nc.gpsimd.tensor_scalar(
                                h2[:],
                                p2g[q][m][:],
                                b2c[:, m : m + 1],
                                0.0,
                                ALU.add,
                                ALU.max,
                            )
                        elif on_dve:
                            D = h2_d_cols
                            r = nc.vector.tensor_scalar(
                                h2f[:, 0:D],
                                p2f[:, 0:D],
                                b2c[:, m : m + 1],
                                0.0,
                                ALU.add,
                                ALU.max,
                            )
                            if D < 1024:
                                r = nc.scalar.activation(
                                    h2f[:, D:],
                                    p2f[:, D:],
                                    AF.Relu,
                                    bias=b2c[:, m : m + 1],
                                )
                        elif unit == 1 and h2_u1_cols:
                            S = h2_u1_cols
                            nc.scalar.activation(
                                h2f[:, 0:S],
                                p2f[:, 0:S],
                                AF.Relu,
                                bias=b2c[:, m : m + 1],
                            )
                            r = nc.vector.tensor_scalar(
                                h2f[:, S:],
                                p2f[:, S:],
                                b2c[:, m : m + 1],
                                0.0,
                                ALU.add,
                                ALU.max,
                            )
                        else:
                            r = nc.scalar.activation(
                                h2[:], p2g[q][m][:], AF.Relu, bias=b2c[:, m : m + 1]
                            )
                        h2g.append(h2)
                        relus.append(r)

                if variant in ("no_l2", "no_l3", "l2_only"):
                    continue

                # layer-3: 8 col-tiled matmuls (2 waves of 4), accumulating
                # into the persistent half-kernel bank.  Fake deps on the
                # group's last relus make the wave schedule-ready together.
                last_of_half = goff == 7 or g == n_groups - 1
                for k in range(2):
                    for j in range(4):
                        q, w = divmod(j, 2)
                        mm = nc.tensor.matmul(
                            ph[32 * j : 32 * j + 32, :],
                            w3s[:, goff, k, :],
                            h2g[2 * q + k][:, w, :],
                            start=(goff == 0 and k == 0),
                            stop=(last_of_half and k == 1),
                            tile_position=(0, 32 * j),
                            skip_group_check=True,
                        )
                        if l3_deps and (k, j) != (1, 3):
                            for r in relus[2:]:
                                add_dep_helper(
                                    mm.ins, r.ins, sync=False,
                                    reason="l3 wave adjacency",
                                )

                # per-half sumexp + diag straight off the PSUM bank (the
                # log of sumexp happens on the host)
                if last_of_half and do_endgame and variant == "full":
                    h = g // 8
                    expt = wpool.tile([128, B], f16, tag="expt", name=f"expt_{h}")
                    nc.scalar.activation(
                        expt[:],
                        ph[:],
                        AF.Exp,
                        accum_out=otile[:, 2 * h : 2 * h + 1],
                    )
                    # diag: one fused op, (ph bypass 1.0) * mask, summed
                    mjunk = wpool.tile([128, B], f32, tag="mjunk", name=f"mj_{h}")
                    nc.vector.scalar_tensor_tensor(
                        mjunk[:],
                        ph[:],
                        1.0,
                        mask[:, h, :],
                        ALU.bypass,
                        ALU.mult,
                        accum_out=otile[:, 2 * h + 1 : 2 * h + 2],
                    )
                    halves.append(h)

            if do_endgame and variant == "full":
                nc.sync.dma_start(out_d[:], otile[:])
            else:
                nc.gpsimd.memset(otile[:], 0.0)
                nc.sync.dma_start(out_d[:], otile[:])


def _make_aps(nc):
    from concourse import mybir

    f32 = mybir.dt.float32
    f32r = mybir.dt.float32r
    f16 = mybir.dt.float16
    f8 = mybir.dt.float8e4

    return {
        "xt": nc.dram_tensor("xt", [NX, B], f32r, kind="ExternalInput").ap(),
        "yta": nc.dram_tensor("yta", [NY + 1, SH], f32r, kind="ExternalInput").ap(),
        "w1x": nc.dram_tensor("w1x", [NX, HID], f32r, kind="ExternalInput").ap(),
        "w1ya": nc.dram_tensor(
            "w1ya", [NY + 1, HID], f32r, kind="ExternalInput"
        ).ap(),
        "w2": nc.dram_tensor("w2", [HID, HID], f8, kind="ExternalInput").ap(),
        "b2": nc.dram_tensor("b2", [HID], f32, kind="ExternalInput").ap(),
        "w3s": nc.dram_tensor("w3s", [128, 2], f16, kind="ExternalInput").ap(),
        "mask": nc.dram_tensor("mask", [128, 1024], f8, kind="ExternalInput").ap(),
        "out": nc.dram_tensor("out", [128, 4], f32, kind="ExternalOutput").ap(),
    }


def _get_program():
    if "nc" in _PROG_CACHE:
        return _PROG_CACHE["nc"]

    import concourse.tile as tile
    from concourse import bacc

    nc = bacc.Bacc(
        "TRN2", target_bir_lowering=False, debug=False, num_devices=N_CORES
    )
    aps = _make_aps(nc)
    with tile.TileContext(nc) as tc:
        _emit(tc, aps)
    nc.compile()

    _PROG_CACHE["nc"] = nc
    return nc


def _sr_quantize_e4m3(x, seed):
    """Stochastic-round x (f32) to fp8 e4m3 (returns ml_dtypes array)."""
    import ml_dtypes

    e4 = ml_dtypes.float8_e4m3
    rng = np.random.default_rng(seed)
    lo = np.asarray(x, np.float32).astype(e4)
    hi = np.nextafter(lo, np.array(np.inf, e4))
    lo32, hi32 = lo.astype(np.float32), hi.astype(np.float32)
    span = np.where(hi32 > lo32, hi32 - lo32, 1.0)
    frac = np.clip((x - lo32) / span, 0.0, 1.0)
    pick_hi = rng.random(x.shape) < frac
    return np.where(pick_hi, hi, lo).astype(e4)


def _make_in_maps(dataX, dataY, W1, b1, W2, b2, W3):
    import ml_dtypes

    e4 = ml_dtypes.float8_e4m3
    dataX = np.asarray(dataX, np.float32)
    dataY = np.asarray(dataY, np.float32)
    W1 = np.asarray(W1, np.float32)
    b1 = np.asarray(b1, np.float32)
    W2 = np.asarray(W2, np.float32)
    b2 = np.asarray(b2, np.float32)
    W3 = np.asarray(W3, np.float32)

    xt = np.ascontiguousarray(dataX.T)
    w1x = np.ascontiguousarray(W1[:NX])
    w1ya = np.ascontiguousarray(np.vstack([W1[NX:], b1[None, :]]))

    # --- host-side fp8 prep: SR quantize W2, compensate b2 ---------------
    w2q = _sr_quantize_e4m3(W2, SR_SEED)
    dW2 = w2q.astype(np.float32) - W2
    Ae = (dataX @ W1[:NX]).astype(np.float16).astype(np.float32)
    C = dataY @ W1[NX:] + b1
    h1bar = np.zeros(HID, np.float64)
    for s in range(0, B, 64):
        blk = np.maximum(Ae[None, :, :] + C[s : s + 64, None, :], 0.0)
        h1bar += blk.astype(e4).astype(np.float32).sum((0, 1), dtype=np.float64)
    h1bar = (h1bar / (B * B)).astype(np.float32)
    b2c = (b2 - dW2.T @ h1bar).astype(np.float32)

    # --- compact w3 chunks [128, 2] f16 (w3s built on device) ------------
    w3s = np.stack(
        [W3[0:128, 0], W3[128:256, 0]], axis=1
    ).astype(np.float16)

    in_maps = []
    for c in range(N_CORES):
        ysh = dataY[c * SH : (c + 1) * SH]
        yta = np.ascontiguousarray(
            np.vstack([ysh.T, np.ones((1, SH), np.float32)])
        )
        # permuted diag mask: partition 32j+gg holds row 32h + 4gg + j
        maskc = np.zeros((128, 2, B), e4)
        for p in range(128):
            j, gg = divmod(p, 32)
            if gg < 8:
                for h in range(2):
                    r = 32 * h + 4 * gg + j
                    maskc[p, h, c * SH + r] = 1.0
        maskc = maskc.reshape(128, 1024)
        in_maps.append(
            {
                "xt": xt,
                "yta": yta,
                "w1x": w1x,
                "w1ya": w1ya,
                "w2": w2q,
                "b2": b2c,
                "w3s": w3s,
                "mask": maskc,
            }
        )
    return in_maps


def _unpermute(out):
    """out [128, 4] (sumexp, diag per half) -> (lse [SH], diag [SH])."""
    lse = np.empty(SH, np.float64)
    diag = np.empty(SH, np.float64)
    for p in range(128):
        j, gg = divmod(p, 32)
        if gg < 8:
            for h in range(2):
                r = 32 * h + 4 * gg + j
                lse[r] = np.log(np.float64(out[p, 2 * h]))
                diag[r] = out[p, 2 * h + 1]
    return lse, diag


def _combine(results):
    parts = [_unpermute(np.asarray(r["out"])) for r in results]
    lse = np.concatenate([p[0] for p in parts])
    diag = np.concatenate([p[1] for p in parts])
    log_b = np.log(np.float64(B))
    mi = log_b + diag.mean() - lse.mean()
    return np.asarray(-mi, dtype=np.float32)


def _run(inputs):
    import time

    from concourse import bass_utils

    nc = _get_program()
    in_maps = _make_in_maps(
        inputs["dataX"],
        inputs["dataY"],
        inputs["W1"],
        inputs["b1"],
        inputs["W2"],
        inputs["b2"],
        inputs["W3"],
    )
    # The axon/NRT path occasionally fails transiently on a fresh session
    # (device-unrecoverable on first touch); retry with backoff.
    last_exc = None
    for attempt in range(4):
        try:
            res = bass_utils.run_bass_kernel_spmd(
                nc, in_maps, core_ids=list(range(N_CORES)), trace=False
            )
            out = _combine(res.results)
            if np.isfinite(out):
                return out, res
            last_exc = RuntimeError("non-finite kernel output")
        except Exception as exc:  # noqa: BLE001
            last_exc = exc
        time.sleep(2.0 * (attempt + 1))
        try:
            import jax

            jax.clear_caches()
        except Exception:  # noqa: BLE001
            pass
    raise last_exc


class _Executor:
    """Reusable sharded executable over the 8 cores, for timing loops."""

    def __init__(self, nc, in_maps):
        import jax
        import numpy as np
        from jax.sharding import Mesh, NamedSharding, PartitionSpec
        from jax.experimental.shard_map import shard_map

        from concourse import bass2jax, mybir

        bass2jax.install_neuronx_cc_hook()

        partition_name = (
            nc.partition_id_tensor.name if nc.partition_id_tensor else None
        )
        in_names, out_names, out_avals, zero_outs = [], [], [], []
        for alloc in nc.m.functions[0].allocations:
            if not isinstance(alloc, mybir.MemoryLocationSet):
                continue
            name = alloc.memorylocations[0].name
            if alloc.kind == "ExternalInput":
                if name != partition_name:
                    in_names.append(name)
            elif alloc.kind == "ExternalOutput":
                out_names.append(name)
                shape = tuple(alloc.tensor_shape)
                dtype = mybir.dt.np(alloc.dtype)
                out_avals.append(jax.core.ShapedArray(shape, dtype))
                zero_outs.append(np.zeros(shape, dtype))
        n_params = len(in_names)
        n_outs = len(out_avals)
        all_in_names = list(in_names) + list(out_names)
        if partition_name is not None:
            all_in_names.append(partition_name)
        donate = tuple(range(n_params, n_params + n_outs))

        def _body(*args):
            operands = list(args)
            if partition_name is not None:
                operands.append(bass2jax.partition_id_tensor())
            outs = bass2jax._bass_exec_p.bind(
                *operands,
                out_avals=tuple(out_avals),
                in_names=tuple(all_in_names),
                out_names=tuple(out_names),
                lowering_input_output_aliases=(),
                sim_require_finite=False,
                sim_require_nnan=False,
                nc=nc,
            )
            return tuple(outs)

        devices = jax.devices()[:N_CORES]
        mesh = Mesh(np.asarray(devices), ("core",))
        in_specs = (PartitionSpec("core"),) * (n_params + n_outs)
        out_specs = (PartitionSpec("core"),) * len(out_names)
        self._fn = jax.jit(
            shard_map(
                _body,
                mesh=mesh,
                in_specs=in_specs,
                out_specs=out_specs,
                check_rep=False,
            ),
            donate_argnums=donate,
            keep_unused=True,
        )
        per_core = [
            [np.asarray(m[name]) for name in in_names] for m in in_maps
        ]
        sharding = NamedSharding(mesh, PartitionSpec("core"))
        self._dev_in = [
            jax.device_put(
                np.concatenate([per_core[c][i] for c in range(N_CORES)], axis=0),
                sharding,
            )
            for i in range(n_params)
        ]
        self._zero_shapes = [
            ((N_CORES * z.shape[0],) + z.shape[1:], z.dtype) for z in zero_outs
        ]
        self._out_names = out_names
        self._out_avals = out_avals
        self._jax = jax

    def __call__(self):
        zeros = [np.zeros(s, d) for s, d in self._zero_shapes]
        outs = self._fn(*self._dev_in, *zeros)
        self._jax.block_until_ready(outs)
        return outs

    def results(self, outs):
        res = []
        for c in range(N_CORES):
            res.append(
                {
                    name: np.asarray(outs[i]).reshape(
                        N_CORES, *self._out_avals[i].shape
                    )[c]
                    for i, name in enumerate(self._out_names)
                }
            )
        return res


def kernel(**inputs):
    return _run(inputs)[0]


# revision 68
# speedup vs baseline: 1.2244x; 1.2244x over previous
"""Trainium2 Bass kernel for the DSIB InfoNCE loss (fp8 DoubleRow version).

Reference computation (B=512, NX=NY=64, HID=256):
    scores[i, j] = MLP(concat(x_j, y_i))       # 3-layer MLP, scalar out
    loss = -(log B + mean(diag(scores)) - mean(logsumexp(scores, axis=1)))

Strategy (data-parallel over the outer y index, 8 cores x 64 rows):
  * Layer 1 is linear in the concatenation: precompute on device
    A = X @ W1[:64] ([512, 256], kept transposed as a16 [128, 2, 512] fp16)
    and C = Y_shard @ W1[64:] + b1 (cb [128, 2, 64] fp32, b1 folded in via
    an augmented ones-row on the host). h1(i, j) = relu(A[j] + C[i]).
  * Layer 2 runs in fp8e4m3 with perf_mode=DoubleRow: h1 is produced
    directly in fp8 as [128, 2, 512] (contraction 256 = 128 partitions x 2
    packed), W2 is stochastically-rounded to fp8 on the host and the
    systematic quantization error is compensated by b2' = b2 - dW2.T @ mean(h1)
    (also computed host-side).  One DoubleRow matmul per (row, out-half)
    does the whole K=256 contraction at 2 MACs/cell/cycle.
  * Layer-2 PSUM output is 2-row batched ([128, 1024] spanning 2 banks) so
    one ACT relu+bias pass covers two rows -> h2 [128, 2, 512] fp16.
  * The relu+bias pass (h2) is the bottleneck (1 elem/lane/cycle from
    PSUM on both ACT and DVE); per group 3 units run on ACT and 1 on DVE
    (empirically balanced against ACT's exp and DVE's h1 work).
  * Layer 3 (fp16) accumulates scores for 8 groups of 4 rows into one
    persistent PSUM bank: for group g the stationary w3 column sits at
    offset g%8 inside a 32-wide zero-padded block at column-group 32j, so
    row 4g+j lands at PSUM partition 32j + g%8.  Scores are never copied
    to SBUF: the per-half sumexp (ACT exp with accum_out; log on host)
    and the masked diagonal extraction (one fused scalar_tensor_tensor
    with accum_out) read the PSUM bank directly.  Each core returns
    [128, 4] = per-partition (sumexp, diag) x 2 halves in the permuted
    layout; the host unpermutes and reduces.
"""

import sys

import numpy as np

_TRN_REPO = "/opt/trn_rl_repo"
if _TRN_REPO not in sys.path:
    sys.path.insert(0, _TRN_REPO)

B = 512
NX = 64
NY = 64
HID = 256
N_CORES = 8
SH = B // N_CORES  # y rows per core
SR_SEED = 0  # host-side stochastic-rounding seed for W2 (validated on HW)

_PROG_CACHE = {}


def _emit(
    tc,
    aps,
    n_rows=SH,
    do_endgame=True,
    repeat=None,
    variant="full",  # full | no_l3 | no_h1 | no_l2 | l2_only
    h1_act_cols=0,  # leading h1 columns computed on ACT instead of DVE
    h2_dve=1,  # how many of the 4 h2 units per group run on DVE vs ACT
    h2_extra_dve=False,  # additionally put unit 1 on DVE for even groups
    h2_qsplit=False,  # split DVE h2 units into [128, 256] quarter ops
    h2_pool=0,  # how many of the 4 h2 units per group run on Pool/GPSIMD
    mm_order="gm",  # "gm": group m-major (4x stationary reuse), "m", "w"
    l3_deps=False,  # fake deps grouping the L3 wave (measured: hurts here)
    h1_ahead=True,  # software-pipeline h1 one group ahead of the DVE h2 unit
    h2_d_cols=768,  # DVE's h2 unit covers cols [0:D]; ACT takes [D:1024]
    h2_u1_cols=0,  # if >0, DVE additionally takes cols [S:1024] of unit 1
):
    _H2_DVE_SETS = {
        0: set(),
        1: {3},
        2: {1, 3},
        3: {1, 2, 3},
        4: {0, 1, 2, 3},
        "u0": {0},
        "u1": {1},
        "u2": {2},
    }
    h2_dve_units = _H2_DVE_SETS[h2_dve]
    _H2_POOL_SETS = {0: set(), 1: {1}, 2: {1, 2}}
    h2_pool_units = _H2_POOL_SETS[h2_pool]
    import contextlib

    import concourse.bass as bass  # noqa: F401
    from concourse import mybir
    from concourse.tile import add_dep_helper

    nc = tc.nc
    f32 = mybir.dt.float32
    f32r = mybir.dt.float32r
    f16 = mybir.dt.float16
    f8 = mybir.dt.float8e4
    AF = mybir.ActivationFunctionType
    ALU = mybir.AluOpType
    AX = mybir.AxisListType
    DR = mybir.MatmulPerfMode.DoubleRow

    xt_d = aps["xt"]
    yta_d = aps["yta"]
    w1x_d = aps["w1x"]
    w1ya_d = aps["w1ya"]
    w2_d = aps["w2"]
    b2_d = aps["b2"]
    w3s_d = aps["w3s"]
    mask_d = aps["mask"]
    out_d = aps["out"]

    assert n_rows % 8 == 0, "row loop works in half-kernels of 8 groups"
    n_groups = n_rows // 4
    rows_per_half = 4 * min(8, n_groups)

    with (
        tc.tile_pool(name="const", bufs=1) as cpool,
        tc.tile_pool(name="work", bufs=2) as wpool,
        tc.tile_pool(name="psum", bufs=3, space="PSUM") as ppool,
        tc.tile_pool(name="psall", bufs=1, space="PSUM") as papool,
    ):
        psall = (
            [
                papool.tile([128, B], f32, tag=f"psall{h}", name=f"psall{h}")
                for h in range(2 if n_groups > 8 else 1)
            ]
            if variant in ("full", "no_h1")
            else []
        )

        # ---------------- persistent loads ----------------
        # Critical-path tensors (layer 1) on the SP HWDGE queue, in order;
        # everything else on the Activation HWDGE queue, so the two rings
        # drain in parallel.
        # layer-1-critical tensors first on the SP HWDGE queue; the compute
        # that needs them is emitted immediately after, so the scheduler
        # splits the DMA-completion semaphores there (emission order sets
        # priority).  Bulk tensors ride the Activation HWDGE ring.
        # xt split across both HWDGE rings to halve the first-DMA wall
        xt = cpool.tile([NX, B], f32r, name="xt_sb")
        nc.sync.dma_start(xt[0:32, :], xt_d[0:32, :])
        nc.scalar.dma_start(xt[32:64, :], xt_d[32:64, :])
        w1x = cpool.tile([NX, HID], f32r, name="w1x_sb")
        nc.sync.dma_start(w1x[:], w1x_d[:])

        # Dummy first ACT op: forces the one-time activation-table load to
        # start at t=0, overlapped with the input DMAs and layer 1.  Exp so
        # the chosen set (exp_and_others) also covers Relu/Identity -> one
        # table load for the whole kernel.
        actwarm = cpool.tile([1, 2], f32, name="actwarm")
        nc.gpsimd.memset(actwarm[:, 0:1], 0.0)
        nc.scalar.activation(actwarm[:, 1:2], actwarm[:, 0:1], AF.Exp)

        # HAM warm-up: ~2us of dummy matmuls on zeros while the input DMAs
        # land, so the PE clock gate reaches 8/8 before layer 1/2 start.
        # They write the psall bank, which the first real L3 matmul clears
        # (start=True), so the garbage never escapes.
        if variant in ("full", "no_h1") and n_groups >= 1:
            zst = cpool.tile([128, 32], f8, name="zstat")
            nc.gpsimd.memset(zst[:], 0.0)
            jmov = cpool.tile([128, B], f8, name="jmov")
            nc.gpsimd.memset(jmov[:], 0.0)
            for _wi in range(5):
                nc.tensor.matmul(
                    psall[0][0:32, :],
                    zst[:],
                    jmov[:],
                    start=True,
                    stop=True,
                    tile_position=(0, 0),
                    skip_group_check=True,
                )

        # ---------------- layer-1 precompute (A side) ----------------
        pa = ppool.tile([128, 2, B], f32, tag="p2", name="pa")
        for m in range(2):
            nc.tensor.matmul(
                pa[:, m, :],
                w1x[:, 128 * m : 128 * m + 128],
                xt[:],
                start=True,
                stop=True,
            )
        # a16 cast split across ACT/DVE so each chunk lands in parallel and
        # the first h1 ops can start on chunk granularity
        a16 = cpool.tile([128, 2, B], f16, name="a16_sb")
        nc.scalar.copy(a16[:, 0, :], pa[:, 0, :])
        nc.vector.tensor_copy(a16[:, 1, :], pa[:, 1, :])

        # ---------------- remaining loads ----------------
        yta = cpool.tile([NY + 1, SH], f32r, name="yta_sb")
        nc.scalar.dma_start(yta[:], yta_d[:])
        w1ya = cpool.tile([NY + 1, HID], f32r, name="w1ya_sb")
        nc.scalar.dma_start(w1ya[:], w1ya_d[:])
        # W2 fp8: (p, k, m) = W2q[k*128 + p, m]
        w2 = cpool.tile([128, 2, HID], f8, name="w2_sb")
        for k in range(2):
            nc.sync.dma_start(w2[:, k, :], w2_d[128 * k : 128 * (k + 1), :])
        b2c = cpool.tile([128, 2], f32, name="b2_sb")
        nc.scalar.dma_start(b2c[:], b2_d.rearrange("(k p) -> p k", p=128))
        # w3s built on device: [128, g, k, 32] fp16, w3 chunk k at col g of
        # block g.  Load the compact [128, 2] w3 chunks, memset the block,
        # then 16 single-column SBUF->SBUF DMA copies (rings are idle).
        w3c = cpool.tile([128, 2], f16, name="w3c_sb")
        nc.scalar.dma_start(w3c[:], w3s_d[:, 0:2])
        w3s = cpool.tile([128, 8, 2, 32], f16, name="w3s_sb")
        nc.gpsimd.memset(w3s[:], 0.0)
        # scatter w3 chunk k to columns g*65 + k*32 (g=0..7) of the flat
        # [128, 512] block in one strided DVE copy per chunk (src g-dim is
        # stride-0)
        for k in range(2):
            src = bass.AP(w3c[:].tensor, k, [[2, 128], [0, 8]])
            dst = bass.AP(w3s[:].tensor, 32 * k, [[512, 128], [65, 8]])
            nc.vector.tensor_copy(dst, src)
        # per-half diag masks in the permuted psall layout (partition 32j+gg
        # holds row 32h + 4gg + j)
        mask = cpool.tile([128, 2, B], f8, name="mask_sb")
        nc.scalar.dma_start(mask[:], mask_d.rearrange("p (h c) -> p h c", h=2))

        # ---------------- layer-1 precompute (C side) ----------------
        pc = ppool.tile([128, 2, B], f32, tag="p2", name="pc")
        for m in range(2):
            nc.tensor.matmul(
                pc[:, m, 0:SH],
                w1ya[:, 128 * m : 128 * m + 128],
                yta[:],
                start=True,
                stop=True,
            )
        cb = cpool.tile([128, 2, SH], f32, name="cb_sb")
        nc.vector.tensor_copy(cb[:, 0, :], pc[:, 0, 0:SH])
        nc.vector.tensor_copy(cb[:, 1, :], pc[:, 1, 0:SH])

        if variant == "no_h1":
            h1d = cpool.tile([128, 2, B], f8, name="h1dummy")
            nc.vector.tensor_copy(h1d[:, 0, :], a16[:, 0, :])
            nc.vector.tensor_copy(h1d[:, 1, :], a16[:, 1, :])

        otile = cpool.tile([128, 4], f32, name="otile")

        # ---------------- main loop ----------------
        loop_cm = (
            tc.For_i(0, repeat, 1)
            if repeat is not None and repeat > 1
            else contextlib.nullcontext()
        )
        SPL = h1_act_cols

        def emit_h1_group(g):
            """h1 tiles for the 4 rows of group g (fp8, DoubleRow layout)."""
            if variant == "no_h1":
                return [h1d] * 4
            hs = []
            for q in range(2):
                for w in range(2):
                    i = 4 * g + 2 * q + w
                    h1 = wpool.tile(
                        [128, 2, B], f8, tag=f"h1_{q}_{w}", name=f"h1_{i}"
                    )
                    for k in range(2):
                        if SPL > 0:
                            nc.scalar.activation(
                                h1[:, k, 0:SPL],
                                a16[:, k, 0:SPL],
                                AF.Relu,
                                bias=cb[:, k, i : i + 1],
                            )
                        if SPL < B:
                            nc.vector.tensor_scalar(
                                h1[:, k, SPL:],
                                a16[:, k, SPL:],
                                cb[:, k, i : i + 1],
                                0.0,
                                ALU.add,
                                ALU.max,
                            )
                    hs.append(h1)
            return hs

        with loop_cm:
            halves = []  # half indices with completed endgame
            h1_next = emit_h1_group(0) if (n_groups and h1_ahead) else None
            for g in range(n_groups):
                ph = psall[g // 8] if psall else None
                goff = g % 8
                h1cur = h1_next if h1_ahead else emit_h1_group(g)
                h2g = []
                relus = []
                p2g = []
                if variant != "no_l2":
                    if mm_order == "gm":
                        # group-level m-major: 4 consecutive matmuls share
                        # each stationary (one LDWEIGHTS run per W2 half)
                        p2g = [
                            [
                                ppool.tile(
                                    [128, 2, B], f32, tag="p2",
                                    name=f"p2_{m}_{4 * g + 2 * q}",
                                )
                                for m in range(2)
                            ]
                            for q in range(2)
                        ]
                        for m in range(2):
                            for q in range(2):
                                for w in range(2):
                                    nc.tensor.matmul(
                                        p2g[q][m][:, w, :],
                                        w2[:, :, 128 * m : 128 * (m + 1)],
                                        h1cur[2 * q + w][:],
                                        start=True,
                                        stop=True,
                                        perf_mode=DR,
                                    )
                    else:
                        for q in range(2):  # row-pair within group
                            pr = 4 * g + 2 * q
                            p2s = [
                                ppool.tile(
                                    [128, 2, B], f32, tag="p2", name=f"p2_{m}_{pr}"
                                )
                                for m in range(2)
                            ]
                            mw = (
                                [(0, 0), (0, 1), (1, 0), (1, 1)]
                                if mm_order == "m"
                                else [(0, 0), (1, 0), (0, 1), (1, 1)]
                            )
                            for m, w in mw:
                                nc.tensor.matmul(
                                    p2s[m][:, w, :],
                                    w2[:, :, 128 * m : 128 * (m + 1)],
                                    h1cur[2 * q + w][:],
                                    start=True,
                                    stop=True,
                                    perf_mode=DR,
                                )
                            p2g.append(p2s)

                # pipelined h1 for the NEXT group: emitted before this
                # group's DVE h2 unit so the (FIFO) DVE stream produces h1
                # ahead of chewing the long PSUM read -- keeps PE/ACT fed.
                if h1_ahead:
                    h1_next = emit_h1_group(g + 1) if g + 1 < n_groups else None

                if variant in ("no_l2", "l2_only"):
                    continue

                for q in range(2):
                    pr = 4 * g + 2 * q
                    for m in range(2):
                        h2 = wpool.tile(
                            [128, 2, B], f16, tag=f"h2_{q}_{m}", name=f"h2_{m}_{pr}"
                        )
                        unit = 2 * q + m
                        on_dve = unit in h2_dve_units or (
                            h2_extra_dve and unit == 1 and g % 2 == 0
                        )
                        h2f = h2[:].rearrange("p a b -> p (a b)")
                        p2f = p2g[q][m][:].rearrange("p a b -> p (a b)")
                        if unit in h2_pool_units and not on_dve:
                            r = nc.gpsimd.tensor_scalar(
                                h2[:],
                                p2g[q][m][:],
                                b2c[:, m : m + 1],
                                0.0,
                                ALU.add,
                                ALU.max,
                            )
                        elif on_dve:
                            D = h2_d_cols
                            r = nc.vector.tensor_scalar(
                                h2f[:, 0:D],
                                p2f[:, 0:D],
                                b2c[:, m : m + 1],
                                0.0,
                                ALU.add,
                                ALU.max,
                            )
                            if D < 1024:
                                r = nc.scalar.activation(
                                    h2f[:, D:],
                                    p2f[:, D:],
                                    AF.Relu,
                                    bias=b2c[:, m : m + 1],
                                )
                        elif unit == 1 and h2_u1_cols:
                            S = h2_u1_cols
                            nc.scalar.activation(
                                h2f[:, 0:S],
                                p2f[:, 0:S],
                                AF.Relu,
                                bias=b2c[:, m : m + 1],
                            )
                            r = nc.vector.tensor_scalar(
                                h2f[:, S:],
                                p2f[:, S:],
                                b2c[:, m : m + 1],
                                0.0,
                                ALU.add,
                                ALU.max,
                            )
                        else:
                            r = nc.scalar.activation(
                                h2[:], p2g[q][m][:], AF.Relu, bias=b2c[:, m : m + 1]
                            )
                        h2g.append(h2)
                        relus.append(r)

                if variant in ("no_l2", "no_l3", "l2_only"):
                    continue

                # layer-3: 8 col-tiled matmuls (2 waves of 4), accumulating
                # into the persistent half-kernel bank.  Fake deps on the
                # group's last relus make the wave schedule-ready together.
                last_of_half = goff == 7 or g == n_groups - 1
                for k in range(2):
                    for j in range(4):
                        q, w = divmod(j, 2)
                        mm = nc.tensor.matmul(
                            ph[32 * j : 32 * j + 32, :],
                            w3s[:, goff, k, :],
                            h2g[2 * q + k][:, w, :],
                            start=(goff == 0 and k == 0),
                            stop=(last_of_half and k == 1),
                            tile_position=(0, 32 * j),
                            skip_group_check=True,
                        )
                        if l3_deps and (k, j) != (1, 3):
                            for r in relus[2:]:
                                add_dep_helper(
                                    mm.ins, r.ins, sync=False,
                                    reason="l3 wave adjacency",
                                )

                # per-half sumexp + diag straight off the PSUM bank (the
                # log of sumexp happens on the host)
                if last_of_half and do_endgame and variant == "full":
                    h = g // 8
                    expt = wpool.tile([128, B], f16, tag="expt", name=f"expt_{h}")
                    nc.scalar.activation(
                        expt[:],
                        ph[:],
                        AF.Exp,
                        accum_out=otile[:, 2 * h : 2 * h + 1],
                    )
                    # diag: one fused op, (ph bypass 1.0) * mask, summed
                    mjunk = wpool.tile([128, B], f32, tag="mjunk", name=f"mj_{h}")
                    nc.vector.scalar_tensor_tensor(
                        mjunk[:],
                        ph[:],
                        1.0,
                        mask[:, h, :],
                        ALU.bypass,
                        ALU.mult,
                        accum_out=otile[:, 2 * h + 1 : 2 * h + 2],
                    )
                    halves.append(h)

            if do_endgame and variant == "full":
                nc.sync.dma_start(out_d[:], otile[:])
            else:
                nc.gpsimd.memset(otile[:], 0.0)
                nc.sync.dma_start(out_d[:], otile[:])


def _make_aps(nc):
    from concourse import mybir

    f32 = mybir.dt.float32
    f32r = mybir.dt.float32r
    f16 = mybir.dt.float16
    f8 = mybir.dt.float8e4

    return {
        "xt": nc.dram_tensor("xt", [NX, B], f32r, kind="ExternalInput").ap(),
        "yta": nc.dram_tensor("yta", [NY + 1, SH], f32r, kind="ExternalInput").ap(),
        "w1x": nc.dram_tensor("w1x", [NX, HID], f32r, kind="ExternalInput").ap(),
        "w1ya": nc.dram_tensor(
            "w1ya", [NY + 1, HID], f32r, kind="ExternalInput"
        ).ap(),
        "w2": nc.dram_tensor("w2", [HID, HID], f8, kind="ExternalInput").ap(),
        "b2": nc.dram_tensor("b2", [HID], f32, kind="ExternalInput").ap(),
        "w3s": nc.dram_tensor("w3s", [128, 2], f16, kind="ExternalInput").ap(),
        "mask": nc.dram_tensor("mask", [128, 1024], f8, kind="ExternalInput").ap(),
        "out": nc.dram_tensor("out", [128, 4], f32, kind="ExternalOutput").ap(),
    }


def _get_program():
    if "nc" in _PROG_CACHE:
        return _PROG_CACHE["nc"]

    import concourse.tile as tile
    from concourse import bacc

    nc = bacc.Bacc(
        "TRN2", target_bir_lowering=False, debug=False, num_devices=N_CORES
    )
    aps = _make_aps(nc)
    with tile.TileContext(nc) as tc:
        _emit(tc, aps)
    nc.compile()

    _PROG_CACHE["nc"] = nc
    return nc


def _sr_quantize_e4m3(x, seed):
    """Stochastic-round x (f32) to fp8 e4m3 (returns ml_dtypes array)."""
    import ml_dtypes

    e4 = ml_dtypes.float8_e4m3
    rng = np.random.default_rng(seed)
    lo = np.asarray(x, np.float32).astype(e4)
    hi = np.nextafter(lo, np.array(np.inf, e4))
    lo32, hi32 = lo.astype(np.float32), hi.astype(np.float32)
    span = np.where(hi32 > lo32, hi32 - lo32, 1.0)
    frac = np.clip((x - lo32) / span, 0.0, 1.0)
    pick_hi = rng.random(x.shape) < frac
    return np.where(pick_hi, hi, lo).astype(e4)


def _make_in_maps(dataX, dataY, W1, b1, W2, b2, W3):
    import ml_dtypes

    e4 = ml_dtypes.float8_e4m3
    dataX = np.asarray(dataX, np.float32)
    dataY = np.asarray(dataY, np.float32)
    W1 = np.asarray(W1, np.float32)
    b1 = np.asarray(b1, np.float32)
    W2 = np.asarray(W2, np.float32)
    b2 = np.asarray(b2, np.float32)
    W3 = np.asarray(W3, np.float32)

    xt = np.ascontiguousarray(dataX.T)
    w1x = np.ascontiguousarray(W1[:NX])
    w1ya = np.ascontiguousarray(np.vstack([W1[NX:], b1[None, :]]))

    # --- host-side fp8 prep: SR quantize W2, compensate b2 ---------------
    w2q = _sr_quantize_e4m3(W2, SR_SEED)
    dW2 = w2q.astype(np.float32) - W2
    Ae = (dataX @ W1[:NX]).astype(np.float16).astype(np.float32)
    C = dataY @ W1[NX:] + b1
    h1bar = np.zeros(HID, np.float64)
    for s in range(0, B, 64):
        blk = np.maximum(Ae[None, :, :] + C[s : s + 64, None, :], 0.0)
        h1bar += blk.astype(e4).astype(np.float32).sum((0, 1), dtype=np.float64)
    h1bar = (h1bar / (B * B)).astype(np.float32)
    b2c = (b2 - dW2.T @ h1bar).astype(np.float32)

    # --- compact w3 chunks [128, 2] f16 (w3s built on device) ------------
    w3s = np.stack(
        [W3[0:128, 0], W3[128:256, 0]], axis=1
    ).astype(np.float16)

    in_maps = []
    for c in range(N_CORES):
        ysh = dataY[c * SH : (c + 1) * SH]
        yta = np.ascontiguousarray(
            np.vstack([ysh.T, np.ones((1, SH), np.float32)])
        )
        # permuted diag mask: partition 32j+gg holds row 32h + 4gg + j
        maskc = np.zeros((128, 2, B), e4)
        for p in range(128):
            j, gg = divmod(p, 32)
            if gg < 8:
                for h in range(2):
                    r = 32 * h + 4 * gg + j
                    maskc[p, h, c * SH + r] = 1.0
        maskc = maskc.reshape(128, 1024)
        in_maps.append(
            {
                "xt": xt,
                "yta": yta,
                "w1x": w1x,
                "w1ya": w1ya,
                "w2": w2q,
                "b2": b2c,
                "w3s": w3s,
                "mask": maskc,
            }
        )
    return in_maps


def _unpermute(out):
    """out [128, 4] (sumexp, diag per half) -> (lse [SH], diag [SH])."""
    lse = np.empty(SH, np.float64)
    diag = np.empty(SH, np.float64)
    for p in range(128):
        j, gg = divmod(p, 32)
        if gg < 8:
            for h in range(2):
                r = 32 * h + 4 * gg + j
                lse[r] = np.log(np.float64(out[p, 2 * h]))
                diag[r] = out[p, 2 * h + 1]
    return lse, diag


def _combine(results):
    parts = [_unpermute(np.asarray(r["out"])) for r in results]
    lse = np.concatenate([p[0] for p in parts])
    diag = np.concatenate([p[1] for p in parts])
    log_b = np.log(np.float64(B))
    mi = log_b + diag.mean() - lse.mean()
    return np.asarray(-mi, dtype=np.float32)


def _run(inputs):
    import time

    from concourse import bass_utils

    nc = _get_program()
    in_maps = _make_in_maps(
        inputs["dataX"],
        inputs["dataY"],
        inputs["W1"],
        inputs["b1"],
        inputs["W2"],
        inputs["b2"],
        inputs["W3"],
    )
    # The axon/NRT path occasionally fails transiently on a fresh session
    # (device-unrecoverable on first touch); retry with backoff.
    last_exc = None
    for attempt in range(4):
        try:
            res = bass_utils.run_bass_kernel_spmd(
                nc, in_maps, core_ids=list(range(N_CORES)), trace=False
            )
            out = _combine(res.results)
            if np.isfinite(out):
                return out, res
            last_exc = RuntimeError("non-finite kernel output")
        except Exception as exc:  # noqa: BLE001
            last_exc = exc
        time.sleep(2.0 * (attempt + 1))
        try:
            import jax

            jax.clear_caches()
        except Exception:  # noqa: BLE001
            pass
    raise last_exc


class _Executor:
    """Reusable sharded executable over the 8 cores, for timing loops."""

    def __init__(self, nc, in_maps):
        import jax
        import numpy as np
        from jax.sharding import Mesh, NamedSharding, PartitionSpec
        from jax.experimental.shard_map import shard_map

        from concourse import bass2jax, mybir

        bass2jax.install_neuronx_cc_hook()

        partition_name = (
            nc.partition_id_tensor.name if nc.partition_id_tensor else None
        )
        in_names, out_names, out_avals, zero_outs = [], [], [], []
        for alloc in nc.m.functions[0].allocations:
            if not isinstance(alloc, mybir.MemoryLocationSet):
                continue
            name = alloc.memorylocations[0].name
            if alloc.kind == "ExternalInput":
                if name != partition_name:
                    in_names.append(name)
            elif alloc.kind == "ExternalOutput":
                out_names.append(name)
                shape = tuple(alloc.tensor_shape)
                dtype = mybir.dt.np(alloc.dtype)
                out_avals.append(jax.core.ShapedArray(shape, dtype))
                zero_outs.append(np.zeros(shape, dtype))
        n_params = len(in_names)
        n_outs = len(out_avals)
        all_in_names = list(in_names) + list(out_names)
        if partition_name is not None:
            all_in_names.append(partition_name)
        donate = tuple(range(n_params, n_params + n_outs))

        def _body(*args):
            operands = list(args)
            if partition_name is not None:
                operands.append(bass2jax.partition_id_tensor())
            outs = bass2jax._bass_exec_p.bind(
                *operands,
                out_avals=tuple(out_avals),
                in_names=tuple(all_in_names),
                out_names=tuple(out_names),
                lowering_input_output_aliases=(),
                sim_require_finite=False,
                sim_require_nnan=False,
                nc=nc,
            )
            return tuple(outs)

        devices = jax.devices()[:N_CORES]
        mesh = Mesh(np.asarray(devices), ("core",))
        in_specs = (PartitionSpec("core"),) * (n_params + n_outs)
        out_specs = (PartitionSpec("core"),) * len(out_names)
        self._fn = jax.jit(
            shard_map(
                _body,
                mesh=mesh,
                in_specs=in_specs,
                out_specs=out_specs,
                check_rep=False,
            ),
            donate_argnums=donate,
            keep_unused=True,
        )
        per_core = [
            [np.asarray(m[name]) for name in in_names] for m in in_maps
        ]
        sharding = NamedSharding(mesh, PartitionSpec("core"))
        self._dev_in = [
            jax.device_put(
                np.concatenate([per_core[c][i] for c in range(N_CORES)], axis=0),
                sharding,
            )
            for i in range(n_params)
        ]
        self._zero_shapes = [
            ((N_CORES * z.shape[0],) + z.shape[1:], z.dtype) for z in zero_outs
        ]
        self._out_names = out_names
        self._out_avals = out_avals
        self._jax = jax

    def __call__(self):
        zeros = [np.zeros(s, d) for s, d in self._zero_shapes]
        outs = self._fn(*self._dev_in, *zeros)
        self._jax.block_until_ready(outs)
        return outs

    def results(self, outs):
        res = []
        for c in range(N_CORES):
            res.append(
                {
                    name: np.asarray(outs[i]).reshape(
                        N_CORES, *self._out_avals[i].shape
                    )[c]
                    for i, name in enumerate(self._out_names)
                }
            )
        return res


def kernel(**inputs):
    return _run(inputs)[0]
